# revision 2
# baseline (speedup 1.0000x reference)
"""Trainium2 Bass kernel for nn_AttentionModule (B=4, S=2048, D=1024, H=16).

Sharding: 8 cores = (batch b = core//2) x (head-group g = core%2, 8 heads each).
Each core computes, for its batch and its 8 heads:
    xp.T = x[b].T + pe.T                              (on device, DVE)
    q.T/k.T = W_loc @ xp  -> [512, 2048]  (i-dims on partitions)
    v      = xp @ Wv_loc.T -> [2048, 512] (tokens on partitions)
    scores_T = k.T.T @ q per (head, key-chunk)  [keys, q]  (PE, heads paired
               in the 128-row array: head dims 0-63 / 64-127)
    P_T = exp(scores/8)  (one ScalarE op per 4-bank psum group)
    ctx.T, denom = [V_h | 1].T @ P_T  (M=65 matmul, psum-accumulated over chunks)
    ctx.T /= denom  (DVE, gpsimd partition_broadcast)
    outT_partial = Wp_loc.T.T @ ctx.T  [1024, 2048]
Host gathers: out[b] = (outT[2b] + outT[2b+1]).T.
All matmuls run in float32r (TF32-like, full PE rate at free-dim >= 256).
"""
import numpy as np
import concourse.mybir as mybir
import concourse.tile as tile
from concourse import bacc, bass_utils

B, S, D, H = 4, 2048, 1024, 16
HD, P = 64, 128
DL = 512            # local qkv dims per core (8 heads)
HL = 8              # local heads per core
NPAIR = 4           # head pairs per core
QB = 512            # query block
NQB = S // QB       # 4
NKC = S // P        # 16 key chunks
DCH = D // P        # 8 d-chunks
VE = HD + 1         # V columns + ones column

f32 = mybir.dt.float32
f32r = mybir.dt.float32r
EXP = mybir.ActivationFunctionType.Exp

_CACHE = {}


def build(with_qkv_bias=False, with_p_bias=False):
    key = (with_qkv_bias, with_p_bias)
    if key in _CACHE:
        return _CACHE[key]

    nc = bacc.Bacc("TRN2", target_bir_lowering=False, debug=False)

    XT = nc.dram_tensor("xT", [D, S], f32, kind="ExternalInput")
    PET = nc.dram_tensor("peT", [D, S], f32, kind="ExternalInput")
    WQT = nc.dram_tensor("wqT", [D, DL], f32, kind="ExternalInput")
    WKT = nc.dram_tensor("wkT", [D, DL], f32, kind="ExternalInput")
    WVT = nc.dram_tensor("wvT", [D, DL], f32, kind="ExternalInput")
    WPT = nc.dram_tensor("wpT", [DL, D], f32, kind="ExternalInput")
    if with_qkv_bias:
        BQ = nc.dram_tensor("bq", [1, DL], f32, kind="ExternalInput")
        BK = nc.dram_tensor("bk", [1, DL], f32, kind="ExternalInput")
        BV = nc.dram_tensor("bv", [1, DL], f32, kind="ExternalInput")
    if with_p_bias:
        BP = nc.dram_tensor("bp", [1, D], f32, kind="ExternalInput")
    OUTT = nc.dram_tensor("outT", [D, S], f32, kind="ExternalOutput")

    with tile.TileContext(nc) as tc:
        with tc.tile_pool(name="ps", bufs=1, space="PSUM") as ps, \
             tc.tile_pool(name="res", bufs=1) as res:
            qT = res.tile([P, NPAIR, S], f32r, name="qT")
            kT = res.tile([P, NPAIR, S], f32r, name="kT")
            vext = res.tile([P, NKC, HL * VE], f32r, name="vext")
            nc.vector.tensor_copy(
                vext[:].rearrange("p c (h e) -> p c h e", e=VE)[:, :, :, HD:VE],
                nc.const_aps.tensor(1.0, (P, NKC, HL, 1), f32),
            )
            if with_qkv_bias or with_p_bias:
                ones_sb = res.tile([1, QB], f32r, name="ones_sb")
                nc.vector.tensor_copy(
                    ones_sb[:], nc.const_aps.tensor(1.0, (1, QB), f32)
                )
            bias_sb = {}
            if with_qkv_bias:
                for nm, t in (("q", BQ), ("k", BK), ("v", BV)):
                    bias_sb[nm] = res.tile([1, DL], f32r, name=f"b{nm}_sb")
                    nc.sync.dma_start(bias_sb[nm][:], t.ap().bitcast(f32r))
            if with_p_bias:
                bias_sb["p"] = res.tile([1, D], f32r, name="bp_sb")
                nc.sync.dma_start(bias_sb["p"][:], BP.ap().bitcast(f32r))

            # ---------------- QKV projection phase ----------------
            with tc.tile_pool(name="qkv", bufs=1) as qp:
                xpT = qp.tile([P, DCH, S], f32r, name="xpT")
                xT_r = XT.ap().rearrange("(c p) t -> p c t", p=P).bitcast(f32r)
                peT_r = PET.ap().rearrange("(c p) t -> p c t", p=P).bitcast(f32r)
                for c in range(DCH):
                    nc.sync.dma_start(xpT[:, c, :], xT_r[:, c, :])
                for c in range(DCH):
                    for hh in range(4):
                        sl = slice(hh * QB, (hh + 1) * QB)
                        pes = qp.tile([P, QB], f32r, tag="pes", bufs=2, name="pes")
                        nc.sync.dma_start(pes[:], peT_r[:, c, sl])
                        nc.vector.tensor_add(xpT[:, c, sl], xpT[:, c, sl], pes[:])

                def qk_proj(W, nm, dst, scale):
                    w = qp.tile([P, DCH, DL], f32r, tag="w", bufs=1, name=f"w{nm}")
                    nc.sync.dma_start(
                        w[:], W.ap().rearrange("(c p) i -> p c i", p=P).bitcast(f32r)
                    )
                    for i in range(NPAIR):
                        for tb in range(NQB):
                            pm = ps.tile([P, QB], f32, tag="mm", bufs=2, name="pm")
                            for c in range(DCH):
                                nc.tensor.matmul(
                                    pm[:],
                                    w[:, c, i * P:(i + 1) * P],
                                    xpT[:, c, tb * QB:(tb + 1) * QB],
                                    start=(c == 0),
                                    stop=(c == DCH - 1 and not with_qkv_bias),
                                )
                            if with_qkv_bias:
                                nc.tensor.matmul(
                                    pm[:],
                                    bias_sb[nm][:, i * P:(i + 1) * P],
                                    ones_sb[:],
                                    start=False, stop=True,
                                )
                            out_sl = dst[:, i, tb * QB:(tb + 1) * QB]
                            if scale is None:
                                nc.vector.tensor_copy(out_sl, pm[:])
                            else:
                                nc.vector.tensor_scalar_mul(out_sl, pm[:], scale)

                qk_proj(WKT, "k", kT, None)

                # V: natural layout [tokens, i]
                wv = qp.tile([P, DCH, DL], f32r, tag="w", bufs=1, name="wv")
                nc.sync.dma_start(
                    wv[:], WVT.ap().rearrange("(c p) i -> p c i", p=P).bitcast(f32r)
                )
                for tt in range(NKC):
                    pv = ps.tile([P, DL], f32, tag="mm", bufs=2, name="pv")
                    for c in range(DCH):
                        nc.tensor.matmul(
                            pv[:],
                            xpT[:, c, tt * P:(tt + 1) * P],
                            wv[:, c, :],
                            start=(c == 0),
                            stop=(c == DCH - 1 and not with_qkv_bias),
                        )
                    if with_qkv_bias:
                        nc.tensor.matmul(
                            pv[:], ones_sb[:, 0:P], bias_sb["v"][:],
                            start=False, stop=True,
                        )
                    nc.vector.tensor_copy(
                        vext[:, tt].rearrange("p (h e) -> p h e", e=VE)[:, :, 0:HD],
                        pv[:].rearrange("p (h d) -> p h d", d=HD),
                    )

                qk_proj(WQT, "q", qT, float(1.0 / np.sqrt(np.float32(HD))))

            # ---------------- attention + out-projection ----------------
            with tc.tile_pool(name="att", bufs=1) as at:
                wpS = at.tile([P, NPAIR, D], f32r, name="wpS")
                nc.sync.dma_start(
                    wpS[:],
                    WPT.ap().rearrange("(c p) o -> p c o", p=P).bitcast(f32r),
                )
                outT_r = OUTT.ap().rearrange("(t p) s -> p t s", p=P)
                for j in range(NQB):
                    jsl = slice(j * QB, (j + 1) * QB)
                    ctx_sbs = []
                    for pr in range(NPAIR):
                        cx = [
                            ps.tile([VE, QB], f32, tag="cx", bufs=2, name=f"cx{hi}")
                            for hi in range(2)
                        ]
                        for g in range(NKC // 2):
                            sc = ps.tile(
                                [P, 2, 2, QB], f32, tag="sc", bufs=1, name="sc"
                            )
                            for ci in range(2):
                                c = 2 * g + ci
                                for hi in range(2):
                                    hsl = slice(hi * HD, (hi + 1) * HD)
                                    nc.tensor.matmul(
                                        sc[:, hi, ci, :],
                                        kT[hsl, pr, c * P:(c + 1) * P],
                                        qT[hsl, pr, jsl],
                                        start=True, stop=True,
                                    )
                            pt = at.tile(
                                [P, 2, 2, QB], f32r, tag="pt", bufs=2, name="pt"
                            )
                            nc.scalar.activation(pt[:], sc[:], EXP)
                            for ci in range(2):
                                c = 2 * g + ci
                                for hi in range(2):
                                    h = 2 * pr + hi
                                    nc.tensor.matmul(
                                        cx[hi][:],
                                        vext[:, c, h * VE:(h + 1) * VE],
                                        pt[:, hi, ci, :],
                                        start=(c == 0), stop=(c == NKC - 1),
                                    )
                        csb = at.tile([P, QB], f32r, tag="csb", bufs=6, name="csb")
                        for hi in range(2):
                            rec = at.tile([1, QB], f32, tag="rec", bufs=2, name="rec")
                            nc.vector.reciprocal(rec[:], cx[hi][HD:VE, :])
                            bc = at.tile([HD, QB], f32, tag="bc", bufs=2, name="bc")
                            nc.gpsimd.partition_broadcast(bc[:], rec[:])
                            nc.vector.tensor_mul(
                                csb[hi * HD:(hi + 1) * HD, :], cx[hi][0:HD, :], bc[:]
                            )
                        ctx_sbs.append(csb)
                    for o in range(D // P):
                        po = ps.tile([P, QB], f32, tag="mm", bufs=2, name="po")
                        for pr in range(NPAIR):
                            nc.tensor.matmul(
                                po[:],
                                wpS[:, pr, o * P:(o + 1) * P],
                                ctx_sbs[pr][:],
                                start=(pr == 0),
                                stop=(pr == NPAIR - 1 and not with_p_bias),
                            )
                        if with_p_bias:
                            nc.tensor.matmul(
                                po[:],
                                bias_sb["p"][:, o * P:(o + 1) * P],
                                ones_sb[:],
                                start=False, stop=True,
                            )
                        osb = at.tile([P, QB], f32, tag="osb", bufs=3, name="osb")
                        nc.vector.tensor_copy(osb[:], po[:])
                        nc.sync.dma_start(outT_r[:, o, jsl], osb[:])

    nc.compile()
    _CACHE[key] = nc
    return nc


def _in_maps(x, Wq, bq, Wk, bk, Wv, bv, Wp, bp, pe, with_qkv_bias, with_p_bias):
    peT = np.ascontiguousarray(pe.T)
    maps = []
    for core in range(8):
        b, g = divmod(core, 2)
        sl = slice(g * DL, (g + 1) * DL)
        m = {
            "xT": np.ascontiguousarray(x[b].T),
            "peT": peT,
            "wqT": np.ascontiguousarray(Wq[sl].T),
            "wkT": np.ascontiguousarray(Wk[sl].T),
            "wvT": np.ascontiguousarray(Wv[sl].T),
            "wpT": np.ascontiguousarray(Wp[:, sl].T),
        }
        if with_qkv_bias:
            m["bq"] = np.ascontiguousarray(bq[sl]).reshape(1, DL)
            m["bk"] = np.ascontiguousarray(bk[sl]).reshape(1, DL)
            m["bv"] = np.ascontiguousarray(bv[sl]).reshape(1, DL)
        if with_p_bias:
            m["bp"] = (bp if g == 0 else np.zeros_like(bp)).reshape(1, D)
        maps.append(m)
    return maps


def run(x, Wq, bq, Wk, bk, Wv, bv, Wp, bp, pe, **spmd_kwargs):
    args = [np.asarray(a, dtype=np.float32) for a in
            (x, Wq, bq, Wk, bk, Wv, bv, Wp, bp, pe)]
    x, Wq, bq, Wk, bk, Wv, bv, Wp, bp, pe = args
    with_qkv_bias = bool(np.any(bq) or np.any(bk) or np.any(bv))
    with_p_bias = bool(np.any(bp))
    nc = build(with_qkv_bias, with_p_bias)
    maps = _in_maps(x, Wq, bq, Wk, bk, Wv, bv, Wp, bp, pe,
                    with_qkv_bias, with_p_bias)
    res = bass_utils.run_bass_kernel_spmd(
        nc, maps, core_ids=list(range(8)), **spmd_kwargs
    )
    out = np.empty((B, S, D), dtype=np.float32)
    for b in range(B):
        out[b] = (res.results[2 * b]["outT"] + res.results[2 * b + 1]["outT"]).T
    return out, res


def kernel(**inputs):
    out, _ = run(**inputs)
    return out


# revision 14
# speedup vs baseline: 1.0414x; 1.0414x over previous
"""Trainium2 Bass kernel for nn_AttentionModule (B=4, S=2048, D=1024, H=16).

Sharding: 8 cores = (batch b = core//2) x (head-group g = core%2, 8 heads each).
Each core computes, for its batch and its 8 heads:
    xp.T = x[b].T + pe.T                              (on device, DVE)
    q.T/k.T = W_loc @ xp  -> [512, 2048]  (i-dims on partitions)
    v      = xp @ Wv_loc.T -> [2048, 512] (tokens on partitions)
    scores_T = k.T.T @ q per (head, key-chunk)  [keys, q]  (PE, heads paired
               in the 128-row array: head dims 0-63 / 64-127)
    P_T = exp(scores/8)  (one ScalarE op per 4-bank psum group)
    ctx.T, denom = [V_h | 1].T @ P_T  (M=65 matmul, psum-accumulated over chunks)
    ctx.T /= denom  (DVE, gpsimd partition_broadcast)
    outT_partial = Wp_loc.T.T @ ctx.T  [1024, 2048]
Host gathers: out[b] = (outT[2b] + outT[2b+1]).T.
All matmuls run in float32r (TF32-like, full PE rate at free-dim >= 256).
"""
import numpy as np
import concourse.mybir as mybir
import concourse.tile as tile
from concourse import bacc, bass_utils

B, S, D, H = 4, 2048, 1024, 16
HD, P = 64, 128
DL = 512            # local qkv dims per core (8 heads)
HL = 8              # local heads per core
NPAIR = 4           # head pairs per core
QB = 512            # query block
NQB = S // QB       # 4
NKC = S // P        # 16 key chunks
DCH = D // P        # 8 d-chunks
VE = HD + 1         # V columns + ones column

f32 = mybir.dt.float32
f32r = mybir.dt.float32r
EXP = mybir.ActivationFunctionType.Exp

_CACHE = {}


def build(with_qkv_bias=False, with_p_bias=False):
    key = (with_qkv_bias, with_p_bias)
    if key in _CACHE:
        return _CACHE[key]

    nc = bacc.Bacc("TRN2", target_bir_lowering=False, debug=False)

    XT = nc.dram_tensor("xT", [D, S], f32, kind="ExternalInput")
    PET = nc.dram_tensor("peT", [D, S], f32, kind="ExternalInput")
    WQT = nc.dram_tensor("wqT", [D, DL], f32, kind="ExternalInput")
    WKT = nc.dram_tensor("wkT", [D, DL], f32, kind="ExternalInput")
    WVT = nc.dram_tensor("wvT", [D, DL], f32, kind="ExternalInput")
    WPT = nc.dram_tensor("wpT", [DL, D], f32, kind="ExternalInput")
    if with_qkv_bias:
        BQ = nc.dram_tensor("bq", [1, DL], f32, kind="ExternalInput")
        BK = nc.dram_tensor("bk", [1, DL], f32, kind="ExternalInput")
        BV = nc.dram_tensor("bv", [1, DL], f32, kind="ExternalInput")
    if with_p_bias:
        BP = nc.dram_tensor("bp", [1, D], f32, kind="ExternalInput")
    OUTT = nc.dram_tensor("outT", [D, S], f32, kind="ExternalOutput")

    with tile.TileContext(nc) as tc:
        with tc.tile_pool(name="res", bufs=1) as res:
            qT = res.tile([P, NPAIR, S], f32r, name="qT")
            kT = res.tile([P, NPAIR, S], f32r, name="kT")
            vext = res.tile([P, NKC, HL * VE], f32r, name="vext")
            nc.vector.tensor_copy(
                vext[:].rearrange("p c (h e) -> p c h e", e=VE)[:, :, :, HD:VE],
                nc.const_aps.tensor(1.0, (P, NKC, HL, 1), f32),
            )
            if with_qkv_bias or with_p_bias:
                ones_sb = res.tile([1, QB], f32r, name="ones_sb")
                nc.vector.tensor_copy(
                    ones_sb[:], nc.const_aps.tensor(1.0, (1, QB), f32)
                )
            bias_sb = {}
            if with_qkv_bias:
                for nm, t in (("q", BQ), ("k", BK), ("v", BV)):
                    bias_sb[nm] = res.tile([1, DL], f32r, name=f"b{nm}_sb")
                    nc.sync.dma_start(bias_sb[nm][:], t.ap().bitcast(f32r))
            if with_p_bias:
                bias_sb["p"] = res.tile([1, D], f32r, name="bp_sb")
                nc.sync.dma_start(bias_sb["p"][:], BP.ap().bitcast(f32r))

            # ---------------- QKV projection phase ----------------
            with tc.tile_pool(name="qkv", bufs=1) as qp, \
                 tc.tile_pool(name="qps", bufs=1, space="PSUM") as ps:
                xT_r = XT.ap().rearrange("(c p) t -> p c t", p=P).bitcast(f32r)
                peT_r = PET.ap().rearrange("(c p) t -> p c t", p=P).bitcast(f32r)
                xpT = []
                for c in range(DCH):
                    xc = qp.tile([P, S], f32r, tag=f"xp{c}", name=f"xp{c}")
                    nc.sync.dma_start(xc[:], xT_r[:, c, :])
                    for hh in range(4):
                        sl = slice(hh * QB, (hh + 1) * QB)
                        pes = qp.tile([P, QB], f32r, tag="pes", bufs=2, name="pes")
                        nc.sync.dma_start(pes[:], peT_r[:, c, sl])
                        nc.vector.tensor_add(xc[:, sl], xc[:, sl], pes[:])
                    xpT.append(xc)

                def load_w_chunk(W, nm, c):
                    wc = qp.tile([P, DL], f32r, tag="w", bufs=4, name=f"w{nm}{c}")
                    nc.sync.dma_start(
                        wc[:],
                        W.ap().rearrange("(c p) i -> p c i", p=P)[:, c, :]
                        .bitcast(f32r),
                    )
                    return wc

                def qk_proj(W, nm, dst, scale, block_major):
                    outer = range(NQB) if block_major else range(NPAIR)
                    inner = range(NPAIR) if block_major else range(NQB)
                    for a in outer:
                        pms = []
                        for b in inner:
                            i, tb = (b, a) if block_major else (a, b)
                            pms.append((i, tb,
                                        ps.tile([P, QB], f32, tag="mm", bufs=8,
                                                name="pm")))
                        for c in range(DCH):
                            wc = load_w_chunk(W, nm, c)
                            for i, tb, pm in pms:
                                nc.tensor.matmul(
                                    pm[:],
                                    wc[:, i * P:(i + 1) * P],
                                    xpT[c][:, tb * QB:(tb + 1) * QB],
                                    start=(c == 0),
                                    stop=(c == DCH - 1 and not with_qkv_bias),
                                )
                        for i, tb, pm in pms:
                            if with_qkv_bias:
                                nc.tensor.matmul(
                                    pm[:],
                                    bias_sb[nm][:, i * P:(i + 1) * P],
                                    ones_sb[:],
                                    start=False, stop=True,
                                )
                            out_sl = dst[:, i, tb * QB:(tb + 1) * QB]
                            if scale is None:
                                nc.vector.tensor_copy(out_sl, pm[:])
                            else:
                                nc.vector.tensor_scalar_mul(out_sl, pm[:], scale)

                # V first (gates all of attention), natural layout [tokens, i]
                for tq in range(NQB):
                    pvs = [ps.tile([P, DL], f32, tag="mm", bufs=8, name="pv")
                           for _ in range(4)]
                    for c in range(DCH):
                        wc = load_w_chunk(WVT, "v", c)
                        for t4, pv in enumerate(pvs):
                            tt = tq * 4 + t4
                            nc.tensor.matmul(
                                pv[:],
                                xpT[c][:, tt * P:(tt + 1) * P],
                                wc[:],
                                start=(c == 0),
                                stop=(c == DCH - 1 and not with_qkv_bias),
                            )
                    for t4, pv in enumerate(pvs):
                        tt = tq * 4 + t4
                        if with_qkv_bias:
                            nc.tensor.matmul(
                                pv[:], ones_sb[:, 0:P], bias_sb["v"][:],
                                start=False, stop=True,
                            )
                        nc.vector.tensor_copy(
                            vext[:, tt].rearrange("p (h e) -> p h e", e=VE)
                            [:, :, 0:HD],
                            pv[:].rearrange("p (h d) -> p h d", d=HD),
                        )

                qk_proj(WKT, "k", kT, None, block_major=False)
                qk_proj(WQT, "q", qT, float(1.0 / np.sqrt(np.float32(HD))),
                        block_major=True)

            # ---------------- attention + out-projection ----------------
            with tc.tile_pool(name="att", bufs=1) as at, \
                 tc.tile_pool(name="aps", bufs=1, space="PSUM") as ps:
                wpS = at.tile([P, NPAIR, D], f32r, name="wpS")
                nc.sync.dma_start(
                    wpS[:],
                    WPT.ap().rearrange("(c p) o -> p c o", p=P).bitcast(f32r),
                )
                outT_r = OUTT.ap().rearrange("(t p) s -> p t s", p=P)
                for j in range(NQB):
                    jsl = slice(j * QB, (j + 1) * QB)
                    ucxs = []
                    for pr in range(NPAIR):
                        cx = [
                            ps.tile([VE, QB], f32, tag="cx", bufs=2, name=f"cx{hi}")
                            for hi in range(2)
                        ]
                        for g in range(NKC // 2):
                            sc = ps.tile(
                                [P, 2, 2, QB], f32, tag="sc", bufs=1, name="sc"
                            )
                            for ci in range(2):
                                c = 2 * g + ci
                                for hi in range(2):
                                    hsl = slice(hi * HD, (hi + 1) * HD)
                                    nc.tensor.matmul(
                                        sc[:, hi, ci, :],
                                        kT[hsl, pr, c * P:(c + 1) * P],
                                        qT[hsl, pr, jsl],
                                        start=True, stop=True,
                                    )
                            pt = at.tile(
                                [P, 2, 2, QB], f32r, tag="pt", bufs=2, name="pt"
                            )
                            nc.scalar.activation(pt[:], sc[:], EXP)
                            for ci in range(2):
                                c = 2 * g + ci
                                for hi in range(2):
                                    h = 2 * pr + hi
                                    nc.tensor.matmul(
                                        cx[hi][:],
                                        vext[:, c, h * VE:(h + 1) * VE],
                                        pt[:, hi, ci, :],
                                        start=(c == 0), stop=(c == NKC - 1),
                                    )
                        # fast psum->sbuf copies (unnormalized) free the cx slots
                        for hi in range(2):
                            ucx = at.tile([VE, QB], f32, tag="ucx", bufs=10,
                                          name="ucx")
                            nc.vector.tensor_copy(ucx[:], cx[hi][:])
                            ucxs.append(ucx)
                    # normalization for all 8 heads of this qblock (off the
                    # PE critical path: reads only sbuf copies)
                    ctx_sbs = []
                    for pr in range(NPAIR):
                        csb = at.tile([P, QB], f32r, tag="csb", bufs=6, name="csb")
                        for hi in range(2):
                            h = 2 * pr + hi
                            rec = at.tile([1, QB], f32, tag="rec", bufs=3,
                                          name="rec")
                            nc.vector.reciprocal(rec[:], ucxs[h][HD:VE, :])
                            bc = at.tile([HD, QB], f32, tag="bc", bufs=3, name="bc")
                            nc.gpsimd.partition_broadcast(bc[:], rec[:])
                            nc.vector.tensor_mul(
                                csb[hi * HD:(hi + 1) * HD, :],
                                ucxs[h][0:HD, :], bc[:],
                            )
                        ctx_sbs.append(csb)
                    for o in range(D // P):
                        po = ps.tile([P, QB], f32, tag="pj", bufs=2, name="po")
                        for pr in range(NPAIR):
                            nc.tensor.matmul(
                                po[:],
                                wpS[:, pr, o * P:(o + 1) * P],
                                ctx_sbs[pr][:],
                                start=(pr == 0),
                                stop=(pr == NPAIR - 1 and not with_p_bias),
                            )
                        if with_p_bias:
                            nc.tensor.matmul(
                                po[:],
                                bias_sb["p"][:, o * P:(o + 1) * P],
                                ones_sb[:],
                                start=False, stop=True,
                            )
                        osb = at.tile([P, QB], f32, tag="osb", bufs=3, name="osb")
                        nc.vector.tensor_copy(osb[:], po[:])
                        nc.sync.dma_start(outT_r[:, o, jsl], osb[:])

    nc.compile()
    _CACHE[key] = nc
    return nc


def _in_maps(x, Wq, bq, Wk, bk, Wv, bv, Wp, bp, pe, with_qkv_bias, with_p_bias):
    peT = np.ascontiguousarray(pe.T)
    maps = []
    for core in range(8):
        b, g = divmod(core, 2)
        sl = slice(g * DL, (g + 1) * DL)
        m = {
            "xT": np.ascontiguousarray(x[b].T),
            "peT": peT,
            "wqT": np.ascontiguousarray(Wq[sl].T),
            "wkT": np.ascontiguousarray(Wk[sl].T),
            "wvT": np.ascontiguousarray(Wv[sl].T),
            "wpT": np.ascontiguousarray(Wp[:, sl].T),
        }
        if with_qkv_bias:
            m["bq"] = np.ascontiguousarray(bq[sl]).reshape(1, DL)
            m["bk"] = np.ascontiguousarray(bk[sl]).reshape(1, DL)
            m["bv"] = np.ascontiguousarray(bv[sl]).reshape(1, DL)
        if with_p_bias:
            m["bp"] = (bp if g == 0 else np.zeros_like(bp)).reshape(1, D)
        maps.append(m)
    return maps


def run(x, Wq, bq, Wk, bk, Wv, bv, Wp, bp, pe, **spmd_kwargs):
    args = [np.asarray(a, dtype=np.float32) for a in
            (x, Wq, bq, Wk, bk, Wv, bv, Wp, bp, pe)]
    x, Wq, bq, Wk, bk, Wv, bv, Wp, bp, pe = args
    with_qkv_bias = bool(np.any(bq) or np.any(bk) or np.any(bv))
    with_p_bias = bool(np.any(bp))
    nc = build(with_qkv_bias, with_p_bias)
    maps = _in_maps(x, Wq, bq, Wk, bk, Wv, bv, Wp, bp, pe,
                    with_qkv_bias, with_p_bias)
    res = bass_utils.run_bass_kernel_spmd(
        nc, maps, core_ids=list(range(8)), **spmd_kwargs
    )
    out = np.empty((B, S, D), dtype=np.float32)
    for b in range(B):
        out[b] = (res.results[2 * b]["outT"] + res.results[2 * b + 1]["outT"]).T
    return out, res


def kernel(**inputs):
    out, _ = run(**inputs)
    return out
